# revision 8
# baseline (speedup 1.0000x reference)
"""Multi-head attention (B=4, S=2048, D=1024, H=16) on 8 TRN2 NeuronCores.

Sharding: core c handles batch b = c//2 and head-half hh = c%2 (8 heads).
Each core computes, for its (b, hh):
    QKV^T projection (feature-major layouts, fp32r matmuls on the PE),
    per-head attention  S^T = K^T.T-style layout so softmax-exp feeds the
    PV matmul directly (no on-chip transpose of the attention matrix),
    denominators via a ones-column appended to V (free on the PE),
    partial output projection against its 512 columns of W_out.
Host sums the two head-half partials per batch and stacks batches.
"""

import os

import numpy as np

B, S, D, H = 4, 2048, 1024, 16
HD = D // H  # 64
NCORES = 8
HEADS_PER_CORE = H // 2  # 8
EC = HEADS_PER_CORE * HD  # 512: per-core span of q / k / v features
E_QK = 2 * EC  # 1024 rows of QKV^T kept feature-major (Q then K)
P = 128

# matmul dtype: "f32r" (full-rate fp32 replicated mode), "f32" (4x slower),
# "bf16" (full rate, lower precision)
MM_DTYPE = os.environ.get("ATTN_MM_DTYPE", "f32r")

_CACHE = {}


def _build_nc(s_len=S):
    import concourse.bacc as bacc
    import concourse.bass as bass
    import concourse.mybir as mybir
    import concourse.tile as tile
    from concourse.masks import make_identity

    f32 = mybir.dt.float32
    mm_dt = {
        "f32r": mybir.dt.float32r,
        "f32": mybir.dt.float32,
        "bf16": mybir.dt.bfloat16,
    }[MM_DTYPE]


    S_CHUNKS = s_len // P  # 16
    PANELS = S_CHUNKS // 2  # 8 panels of 256 sequence positions
    QS = s_len // 512  # 4 q-slices of 512
    KC = S_CHUNKS  # 16 k-chunks of 128
    DC = D // P  # 8 contraction chunks for the projections
    ODC = (4 * EC) // P // 4  # 4 contraction chunks (of 128) for out proj

    nc = bacc.Bacc("TRN2", target_bir_lowering=False, debug=False)

    x_ap = nc.dram_tensor("x_b", [s_len, D], f32, kind="ExternalInput").ap()
    wqkv_ap = nc.dram_tensor("w_qkv_s", [3 * EC, D], f32, kind="ExternalInput").ap()
    bqkv_ap = nc.dram_tensor("b_qkv_s", [3 * EC], f32, kind="ExternalInput").ap()
    wout_ap = nc.dram_tensor("w_out_s", [D, EC], f32, kind="ExternalInput").ap()
    bout_ap = nc.dram_tensor("b_out", [D], f32, kind="ExternalInput").ap()
    out_ap = nc.dram_tensor("out_part", [s_len, D], f32, kind="ExternalOutput").ap()

    with tile.TileContext(nc) as tc:
        with tc.tile_pool(name="persist", bufs=1) as persist:
            # persistent SBUF tensors
            qkt = persist.tile([P, DC, s_len], mm_dt)  # QK^T feature-major
            vnat = persist.tile([P, S_CHUNKS, HEADS_PER_CORE, HD + 1], mm_dt)
            ident = persist.tile([P, P], f32)
            bqk = persist.tile([P, DC], f32)  # q/k bias, feature on partition
            bv = persist.tile([P, EC], f32)  # v bias broadcast over partitions
            bout = persist.tile([P, D], f32)  # out bias broadcast over partitions
            wout_t = persist.tile([P, ODC, D], mm_dt)

            ones1 = persist.tile([P, 1], f32)
            make_identity(nc, ident)
            nc.vector.memset(ones1, 1.0)
            for sc in range(S_CHUNKS):
                nc.vector.tensor_copy(
                    vnat[:, sc, :, HD], ones1.to_broadcast((P, HEADS_PER_CORE))
                )
            nc.sync.dma_start(bqk, bqkv_ap[0:E_QK].rearrange("(o p) -> p o", p=P))
            nc.sync.dma_start(bv, bqkv_ap[None, E_QK : 3 * EC].to_broadcast((P, EC)))
            nc.sync.dma_start(bout, bout_ap[None, :].to_broadcast((P, D)))

            # ---- Phase A: transpose weights + x panels, project QKV ----
            with (
                tc.tile_pool(name="wq_t", bufs=1) as wq_pool,
                tc.tile_pool(name="stage", bufs=3) as stage,
                tc.tile_pool(name="xt", bufs=2) as xt_pool,
                tc.tile_pool(name="tp_ps", bufs=2, space="PSUM") as tp_ps,
                tc.tile_pool(name="qk_ps", bufs=3, space="PSUM") as qk_ps,
                tc.tile_pool(name="v_ps", bufs=2, space="PSUM") as v_ps,
            ):
                wqkv_t = wq_pool.tile([P, DC, 3 * EC], mm_dt)
                # W_qkv slice: [1536, 1024] -> W^T [128, 8, 1536]
                for ec in range(3 * EC // P):  # 12
                    wst = stage.tile([P, D], f32, tag="stage")
                    nc.sync.dma_start(wst, wqkv_ap[ec * P : (ec + 1) * P, :])
                    for dc in range(DC):
                        pt = tp_ps.tile([P, P], f32)
                        nc.tensor.transpose(pt, wst[:, dc * P : (dc + 1) * P], ident)
                        nc.vector.tensor_copy(
                            wqkv_t[:, dc, ec * P : (ec + 1) * P], pt
                        )

                for panel in range(PANELS):
                    xt = xt_pool.tile([P, DC, 256], mm_dt, tag="xt")
                    for sc_in in range(2):
                        sc = panel * 2 + sc_in
                        xst = stage.tile([P, D], f32, tag="stage")
                        nc.sync.dma_start(xst, x_ap[sc * P : (sc + 1) * P, :])
                        for dc in range(DC):
                            pt = tp_ps.tile([P, P], f32)
                            nc.tensor.transpose(
                                pt, xst[:, dc * P : (dc + 1) * P], ident
                            )
                            nc.vector.tensor_copy(
                                xt[:, dc, sc_in * P : (sc_in + 1) * P], pt
                            )
                    # Q,K feature-major projection: psum [e-chunk 128, s 256]
                    for ec in range(E_QK // P):  # 8
                        ps = qk_ps.tile([P, 256], f32)
                        for dc in range(DC):
                            nc.tensor.matmul(
                                ps,
                                (wqkv_t[:, dc, ec * P : (ec + 1) * P]),
                                (xt[:, dc, :]),
                                start=(dc == 0),
                                stop=(dc == DC - 1),
                            )
                        nc.vector.tensor_scalar_add(
                            qkt[:, ec, panel * 256 : (panel + 1) * 256],
                            ps,
                            bqk[:, ec : ec + 1],
                        )
                    # V natural: psum [s 128, v 512]
                    for sc_in in range(2):
                        sc = panel * 2 + sc_in
                        ps = v_ps.tile([P, EC], f32)
                        for dc in range(DC):
                            nc.tensor.matmul(
                                ps,
                                (xt[:, dc, sc_in * P : (sc_in + 1) * P]),
                                (wqkv_t[:, dc, E_QK : 3 * EC]),
                                start=(dc == 0),
                                stop=(dc == DC - 1),
                            )
                        for h in range(HEADS_PER_CORE):
                            nc.vector.tensor_add(
                                out=vnat[:, sc, h, 0:HD],
                                in0=ps[:, h * HD : (h + 1) * HD],
                                in1=bv[:, h * HD : (h + 1) * HD],
                            )

            # ---- Phase B: transpose W_out slice -> [128, 4, 1024] ----
            with (
                tc.tile_pool(name="wo_stage", bufs=2) as wo_stage,
                tc.tile_pool(name="tp_ps2", bufs=2, space="PSUM") as tp_ps2,
            ):
                for ec in range(D // P):  # 8
                    wst = wo_stage.tile([P, EC], f32, tag="wo")
                    nc.sync.dma_start(wst, wout_ap[ec * P : (ec + 1) * P, :])
                    for dc in range(ODC):
                        pt = tp_ps2.tile([P, P], f32)
                        nc.tensor.transpose(pt, wst[:, dc * P : (dc + 1) * P], ident)
                        nc.vector.tensor_copy(wout_t[:, dc, ec * P : (ec + 1) * P], pt)

            # ---- Phase C: attention ----
            with tc.tile_pool(name="vt", bufs=1) as vt_pool:
              vt = vt_pool.tile([P, ODC, s_len], mm_dt)  # values^T, d-major
              with (
                tc.tile_pool(name="et", bufs=3) as et_pool,
                tc.tile_pool(name="rb", bufs=2) as rb_pool,
                tc.tile_pool(name="vtt", bufs=2) as vtt_pool,
                tc.tile_pool(name="sg_ps", bufs=2, space="PSUM") as sg_ps,
                tc.tile_pool(name="pv_ps", bufs=2, space="PSUM") as pv_ps,
              ):
                for h in range(HEADS_PER_CORE):
                    p0 = (h % 2) * HD
                    qrow = h // 2
                    krow = E_QK // P // 2 + h // 2  # K chunks come after Q chunks
                    for qs in range(QS):
                        pv = pv_ps.tile([HD + 1, 512], f32)
                        for kg in range(KC // 2):
                            sg = sg_ps.tile([P, 2, 512], f32)
                            et = et_pool.tile([P, 2, 512], mm_dt, tag="et")
                            for sub in range(2):
                                kc = kg * 2 + sub
                                nc.tensor.matmul(
                                    sg[:, sub, :],
                                    (qkt[p0 : p0 + HD, krow, kc * P : (kc + 1) * P]),
                                    (qkt[p0 : p0 + HD, qrow, qs * 512 : (qs + 1) * 512]),
                                    start=True,
                                    stop=True,
                                )
                            nc.scalar.activation(
                                et,
                                sg,
                                mybir.ActivationFunctionType.Exp,
                                scale=float(1.0 / np.sqrt(HD)),
                            )
                            for sub in range(2):
                                kc = kg * 2 + sub
                                nc.tensor.matmul(
                                    pv,
                                    (vnat[:, kc, h, :]),
                                    (et[:, sub, :]),
                                    start=(kc == 0),
                                    stop=(kc == KC - 1),
                                )
                        # normalize: row HD of pv is the softmax denominator
                        rb = rb_pool.tile([HD + 1, 512], f32, tag="rb")
                        nc.vector.reciprocal(rb[HD : HD + 1, :], pv[HD : HD + 1, :])
                        nc.sync.dma_start(
                            rb[0:HD, :],
                            rb[HD : HD + 1, None, :].to_broadcast((1, HD, 512)),
                        )
                        if h % 2 == 0:
                            nc.vector.tensor_mul(
                                out=vt[0:HD, h // 2, qs * 512 : (qs + 1) * 512],
                                in0=pv[0:HD, :],
                                in1=rb[0:HD, :],
                            )
                        else:
                            vtt = vtt_pool.tile([HD, 512], mm_dt, tag="vtt")
                            nc.vector.tensor_mul(
                                out=vtt, in0=pv[0:HD, :], in1=rb[0:HD, :]
                            )
                            nc.sync.dma_start(
                                vt[HD : 2 * HD, h // 2, qs * 512 : (qs + 1) * 512],
                                vtt,
                            )

              # ---- Phase D: output projection ----
              with (
                    tc.tile_pool(name="osb", bufs=3) as osb_pool,
                    tc.tile_pool(name="op_ps", bufs=3, space="PSUM") as op_ps,
              ):
                    for sc in range(S_CHUNKS):
                        for eh in range(2):
                            ps = op_ps.tile([P, 512], f32)
                            for dc in range(ODC):
                                nc.tensor.matmul(
                                    ps,
                                    (vt[:, dc, sc * P : (sc + 1) * P]),
                                    (wout_t[:, dc, eh * 512 : (eh + 1) * 512]),
                                    start=(dc == 0),
                                    stop=(dc == ODC - 1),
                                )
                            osb = osb_pool.tile([P, 512], f32, tag="osb")
                            nc.vector.tensor_add(
                                out=osb,
                                in0=ps,
                                in1=bout[:, eh * 512 : (eh + 1) * 512],
                            )
                            nc.sync.dma_start(
                                out_ap[
                                    sc * P : (sc + 1) * P, eh * 512 : (eh + 1) * 512
                                ],
                                osb,
                            )

    nc.compile()
    return nc


def _shard(inputs, s_len=S):
    x = np.ascontiguousarray(np.asarray(inputs["x"], dtype=np.float32))
    wqkv = np.asarray(inputs["W_qkv"], dtype=np.float32)
    bqkv = np.asarray(inputs["b_qkv"], dtype=np.float32)
    wout = np.asarray(inputs["W_out"], dtype=np.float32)
    bout = np.asarray(inputs["b_out"], dtype=np.float32)
    zeros_bout = np.zeros_like(bout)

    in_maps = []
    for c in range(NCORES):
        b, hh = c // 2, c % 2
        heads = range(hh * HEADS_PER_CORE, (hh + 1) * HEADS_PER_CORE)
        # reference reshapes qkv to [s, H, 3*HD]: head h owns rows
        # [h*192, h*192+192) of W_qkv as (q 64 | k 64 | v 64)
        rows = np.concatenate(
            [np.arange(h * 3 * HD + t * HD, h * 3 * HD + (t + 1) * HD)
             for t in range(3) for h in heads]
        )
        in_maps.append(
            {
                "x_b": np.ascontiguousarray(x[b, :s_len]),
                "w_qkv_s": np.ascontiguousarray(wqkv[rows]),
                "b_qkv_s": np.ascontiguousarray(bqkv[rows]),
                "w_out_s": np.ascontiguousarray(wout[:, hh * EC : (hh + 1) * EC]),
                "b_out": bout if hh == 0 else zeros_bout,
            }
        )
    return in_maps


def kernel(**inputs) -> np.ndarray:
    from concourse.bass_utils import run_bass_kernel_spmd

    if "nc" not in _CACHE:
        _CACHE["nc"] = _build_nc()
    nc = _CACHE["nc"]

    in_maps = _shard(inputs)
    trace = os.environ.get("ATTN_TRACE", "0") == "1"
    res = run_bass_kernel_spmd(
        nc, in_maps, core_ids=list(range(NCORES)), trace=trace
    )
    _CACHE["last_results"] = res

    out = np.empty((B, S, D), dtype=np.float32)
    for b in range(B):
        out[b] = res.results[2 * b]["out_part"] + res.results[2 * b + 1]["out_part"]
    return out


# revision 11
# speedup vs baseline: 1.0023x; 1.0023x over previous
"""Multi-head attention (B=4, S=2048, D=1024, H=16) on 8 TRN2 NeuronCores.

Sharding: core c handles batch b = c//2 and head-half hh = c%2 (8 heads).
Each core computes, for its (b, hh):
    QKV^T projection (feature-major layouts, fp32r matmuls on the PE),
    per-head attention  S^T = K^T.T-style layout so softmax-exp feeds the
    PV matmul directly (no on-chip transpose of the attention matrix),
    denominators via a ones-column appended to V (free on the PE),
    partial output projection against its 512 columns of W_out.
Host sums the two head-half partials per batch and stacks batches.
"""

import os

import numpy as np

B, S, D, H = 4, 2048, 1024, 16
HD = D // H  # 64
NCORES = 8
HEADS_PER_CORE = H // 2  # 8
EC = HEADS_PER_CORE * HD  # 512: per-core span of q / k / v features
E_QK = 2 * EC  # 1024 rows of QKV^T kept feature-major (Q then K)
P = 128

# matmul dtype: "f32r" (full-rate fp32 replicated mode), "f32" (4x slower),
# "bf16" (full rate, lower precision)
MM_DTYPE = os.environ.get("ATTN_MM_DTYPE", "f32r")

_CACHE = {}


def _build_nc(s_len=S):
    import concourse.bacc as bacc
    import concourse.bass as bass
    import concourse.mybir as mybir
    import concourse.tile as tile
    from concourse.masks import make_identity

    f32 = mybir.dt.float32
    mm_dt = {
        "f32r": mybir.dt.float32r,
        "f32": mybir.dt.float32,
        "bf16": mybir.dt.bfloat16,
    }[MM_DTYPE]
    # attention (S^T / PV) matmul dtype: partial-array shapes (K=64, M=65)
    # run fp32r at ~2.5 cyc/row on HW, while fp16 streams at 1 cyc/row
    at_dt = {
        "f16": mybir.dt.float16,
        "bf16": mybir.dt.bfloat16,
        "f32r": mybir.dt.float32r,
        "f32": mybir.dt.float32,
    }[os.environ.get("ATTN_ATTN_DT", "f16")]


    S_CHUNKS = s_len // P  # 16
    PANELS = S_CHUNKS // 2  # 8 panels of 256 sequence positions
    QS = s_len // 512  # 4 q-slices of 512
    KC = S_CHUNKS  # 16 k-chunks of 128
    DC = D // P  # 8 contraction chunks for the projections
    ODC = (4 * EC) // P // 4  # 4 contraction chunks (of 128) for out proj

    nc = bacc.Bacc("TRN2", target_bir_lowering=False, debug=False)

    x_ap = nc.dram_tensor("x_b", [s_len, D], f32, kind="ExternalInput").ap()
    wqkv_ap = nc.dram_tensor("w_qkv_s", [3 * EC, D], f32, kind="ExternalInput").ap()
    bqkv_ap = nc.dram_tensor("b_qkv_s", [3 * EC], f32, kind="ExternalInput").ap()
    wout_ap = nc.dram_tensor("w_out_s", [D, EC], f32, kind="ExternalInput").ap()
    bout_ap = nc.dram_tensor("b_out", [D], f32, kind="ExternalInput").ap()
    out_ap = nc.dram_tensor("out_part", [s_len, D], f32, kind="ExternalOutput").ap()

    with tile.TileContext(nc) as tc:
        with tc.tile_pool(name="persist", bufs=1) as persist:
            # persistent SBUF tensors
            qkt = persist.tile([P, DC, s_len], at_dt)  # QK^T feature-major
            vnat = persist.tile([P, S_CHUNKS, HEADS_PER_CORE, HD + 1], at_dt)
            ident = persist.tile([P, P], f32)
            bqk = persist.tile([P, DC], f32)  # q/k bias, feature on partition
            bv = persist.tile([P, EC], f32)  # v bias broadcast over partitions
            bout = persist.tile([P, D], f32)  # out bias broadcast over partitions
            wout_t = persist.tile([P, ODC, D], mm_dt)

            ones1 = persist.tile([P, 1], f32)
            make_identity(nc, ident)
            nc.vector.memset(ones1, 1.0)
            for sc in range(S_CHUNKS):
                nc.vector.tensor_copy(
                    vnat[:, sc, :, HD], ones1.to_broadcast((P, HEADS_PER_CORE))
                )
            nc.sync.dma_start(bqk, bqkv_ap[0:E_QK].rearrange("(o p) -> p o", p=P))
            nc.sync.dma_start(bv, bqkv_ap[None, E_QK : 3 * EC].to_broadcast((P, EC)))
            nc.sync.dma_start(bout, bout_ap[None, :].to_broadcast((P, D)))

            # ---- Phase A: transpose weights + x panels, project QKV ----
            with (
                tc.tile_pool(name="wq_t", bufs=1) as wq_pool,
                tc.tile_pool(name="stage", bufs=3) as stage,
                tc.tile_pool(name="xt", bufs=2) as xt_pool,
                tc.tile_pool(name="tp_ps", bufs=2, space="PSUM") as tp_ps,
                tc.tile_pool(name="qk_ps", bufs=3, space="PSUM") as qk_ps,
                tc.tile_pool(name="v_ps", bufs=2, space="PSUM") as v_ps,
            ):
                wqkv_t = wq_pool.tile([P, DC, 3 * EC], mm_dt)
                # W_qkv slice: [1536, 1024] -> W^T [128, 8, 1536]
                for ec in range(3 * EC // P):  # 12
                    wst = stage.tile([P, D], f32, tag="stage")
                    nc.sync.dma_start(wst, wqkv_ap[ec * P : (ec + 1) * P, :])
                    for dc in range(DC):
                        pt = tp_ps.tile([P, P], f32)
                        nc.tensor.transpose(pt, wst[:, dc * P : (dc + 1) * P], ident)
                        nc.vector.tensor_copy(
                            wqkv_t[:, dc, ec * P : (ec + 1) * P], pt
                        )

                for panel in range(PANELS):
                    xt = xt_pool.tile([P, DC, 256], mm_dt, tag="xt")
                    for sc_in in range(2):
                        sc = panel * 2 + sc_in
                        xst = stage.tile([P, D], f32, tag="stage")
                        nc.sync.dma_start(xst, x_ap[sc * P : (sc + 1) * P, :])
                        for dc in range(DC):
                            pt = tp_ps.tile([P, P], f32)
                            nc.tensor.transpose(
                                pt, xst[:, dc * P : (dc + 1) * P], ident
                            )
                            nc.vector.tensor_copy(
                                xt[:, dc, sc_in * P : (sc_in + 1) * P], pt
                            )
                    # Q,K feature-major projection: psum [e-chunk 128, s 256]
                    for ec in range(E_QK // P):  # 8
                        ps = qk_ps.tile([P, 256], f32)
                        for dc in range(DC):
                            nc.tensor.matmul(
                                ps,
                                (wqkv_t[:, dc, ec * P : (ec + 1) * P]),
                                (xt[:, dc, :]),
                                start=(dc == 0),
                                stop=(dc == DC - 1),
                            )
                        nc.vector.tensor_scalar_add(
                            qkt[:, ec, panel * 256 : (panel + 1) * 256],
                            ps,
                            bqk[:, ec : ec + 1],
                        )
                    # V natural: psum [s 128, v 512]
                    for sc_in in range(2):
                        sc = panel * 2 + sc_in
                        ps = v_ps.tile([P, EC], f32)
                        for dc in range(DC):
                            nc.tensor.matmul(
                                ps,
                                (xt[:, dc, sc_in * P : (sc_in + 1) * P]),
                                (wqkv_t[:, dc, E_QK : 3 * EC]),
                                start=(dc == 0),
                                stop=(dc == DC - 1),
                            )
                        for h in range(HEADS_PER_CORE):
                            nc.vector.tensor_add(
                                out=vnat[:, sc, h, 0:HD],
                                in0=ps[:, h * HD : (h + 1) * HD],
                                in1=bv[:, h * HD : (h + 1) * HD],
                            )

            # ---- Phase B: transpose W_out slice -> [128, 4, 1024] ----
            with (
                tc.tile_pool(name="wo_stage", bufs=2) as wo_stage,
                tc.tile_pool(name="tp_ps2", bufs=2, space="PSUM") as tp_ps2,
            ):
                for ec in range(D // P):  # 8
                    wst = wo_stage.tile([P, EC], f32, tag="wo")
                    nc.sync.dma_start(wst, wout_ap[ec * P : (ec + 1) * P, :])
                    for dc in range(ODC):
                        pt = tp_ps2.tile([P, P], f32)
                        nc.tensor.transpose(pt, wst[:, dc * P : (dc + 1) * P], ident)
                        nc.vector.tensor_copy(wout_t[:, dc, ec * P : (ec + 1) * P], pt)

            # ---- Phase C: attention ----
            with tc.tile_pool(name="vt", bufs=1) as vt_pool:
              vt = vt_pool.tile([P, ODC, s_len], mm_dt)  # values^T, d-major
              with (
                tc.tile_pool(name="et", bufs=3) as et_pool,
                tc.tile_pool(name="rb", bufs=2) as rb_pool,
                tc.tile_pool(name="vtt", bufs=2) as vtt_pool,
                tc.tile_pool(name="sg_ps", bufs=2, space="PSUM") as sg_ps,
                tc.tile_pool(name="pv_ps", bufs=2, space="PSUM") as pv_ps,
              ):
                for h in range(HEADS_PER_CORE):
                    p0 = (h % 2) * HD
                    qrow = h // 2
                    krow = E_QK // P // 2 + h // 2  # K chunks come after Q chunks
                    for qs in range(QS):
                        pv = pv_ps.tile([HD + 1, 512], f32)

                        def pv_mms(kg, ets):
                            # deferred PV for group kg, after S^T of kg+1 has
                            # been emitted — keeps the in-order PE stream from
                            # stalling on the ACT exp and cooling the clock
                            for sub in range(2):
                                kc = kg * 2 + sub
                                nc.tensor.matmul(
                                    pv,
                                    (vnat[:, kc, h, :]),
                                    (ets[kg][:, sub, :]),
                                    start=(kc == 0),
                                    stop=(kc == KC - 1),
                                )

                        ets = {}
                        for kg in range(KC // 2):
                            sg = sg_ps.tile([P, 2, 512], f32)
                            ets[kg] = et_pool.tile(
                                [P, 2, 512], at_dt, tag="et", name="et"
                            )
                            for sub in range(2):
                                kc = kg * 2 + sub
                                nc.tensor.matmul(
                                    sg[:, sub, :],
                                    (qkt[p0 : p0 + HD, krow, kc * P : (kc + 1) * P]),
                                    (qkt[p0 : p0 + HD, qrow, qs * 512 : (qs + 1) * 512]),
                                    start=True,
                                    stop=True,
                                )
                            nc.scalar.activation(
                                ets[kg],
                                sg,
                                mybir.ActivationFunctionType.Exp,
                                scale=float(1.0 / np.sqrt(HD)),
                            )
                            if kg >= 1:
                                pv_mms(kg - 1, ets)
                        pv_mms(KC // 2 - 1, ets)
                        # normalize: row HD of pv is the softmax denominator
                        rb = rb_pool.tile([HD + 1, 512], f32, tag="rb")
                        nc.vector.reciprocal(rb[HD : HD + 1, :], pv[HD : HD + 1, :])
                        nc.sync.dma_start(
                            rb[0:HD, :],
                            rb[HD : HD + 1, None, :].to_broadcast((1, HD, 512)),
                        )
                        if h % 2 == 0:
                            nc.vector.tensor_mul(
                                out=vt[0:HD, h // 2, qs * 512 : (qs + 1) * 512],
                                in0=pv[0:HD, :],
                                in1=rb[0:HD, :],
                            )
                        else:
                            vtt = vtt_pool.tile([HD, 512], mm_dt, tag="vtt")
                            nc.vector.tensor_mul(
                                out=vtt, in0=pv[0:HD, :], in1=rb[0:HD, :]
                            )
                            nc.sync.dma_start(
                                vt[HD : 2 * HD, h // 2, qs * 512 : (qs + 1) * 512],
                                vtt,
                            )

              # ---- Phase D: output projection ----
              with (
                    tc.tile_pool(name="osb", bufs=3) as osb_pool,
                    tc.tile_pool(name="op_ps", bufs=3, space="PSUM") as op_ps,
              ):
                    for sc in range(S_CHUNKS):
                        for eh in range(2):
                            ps = op_ps.tile([P, 512], f32)
                            for dc in range(ODC):
                                nc.tensor.matmul(
                                    ps,
                                    (vt[:, dc, sc * P : (sc + 1) * P]),
                                    (wout_t[:, dc, eh * 512 : (eh + 1) * 512]),
                                    start=(dc == 0),
                                    stop=(dc == ODC - 1),
                                )
                            osb = osb_pool.tile([P, 512], f32, tag="osb")
                            nc.vector.tensor_add(
                                out=osb,
                                in0=ps,
                                in1=bout[:, eh * 512 : (eh + 1) * 512],
                            )
                            nc.sync.dma_start(
                                out_ap[
                                    sc * P : (sc + 1) * P, eh * 512 : (eh + 1) * 512
                                ],
                                osb,
                            )

    nc.compile()
    return nc


def _shard(inputs, s_len=S):
    x = np.ascontiguousarray(np.asarray(inputs["x"], dtype=np.float32))
    wqkv = np.asarray(inputs["W_qkv"], dtype=np.float32)
    bqkv = np.asarray(inputs["b_qkv"], dtype=np.float32)
    wout = np.asarray(inputs["W_out"], dtype=np.float32)
    bout = np.asarray(inputs["b_out"], dtype=np.float32)
    zeros_bout = np.zeros_like(bout)

    in_maps = []
    for c in range(NCORES):
        b, hh = c // 2, c % 2
        heads = range(hh * HEADS_PER_CORE, (hh + 1) * HEADS_PER_CORE)
        # reference reshapes qkv to [s, H, 3*HD]: head h owns rows
        # [h*192, h*192+192) of W_qkv as (q 64 | k 64 | v 64)
        rows = np.concatenate(
            [np.arange(h * 3 * HD + t * HD, h * 3 * HD + (t + 1) * HD)
             for t in range(3) for h in heads]
        )
        in_maps.append(
            {
                "x_b": np.ascontiguousarray(x[b, :s_len]),
                "w_qkv_s": np.ascontiguousarray(wqkv[rows]),
                "b_qkv_s": np.ascontiguousarray(bqkv[rows]),
                "w_out_s": np.ascontiguousarray(wout[:, hh * EC : (hh + 1) * EC]),
                "b_out": bout if hh == 0 else zeros_bout,
            }
        )
    return in_maps


def kernel(**inputs) -> np.ndarray:
    from concourse.bass_utils import run_bass_kernel_spmd

    if "nc" not in _CACHE:
        _CACHE["nc"] = _build_nc()
    nc = _CACHE["nc"]

    in_maps = _shard(inputs)
    trace = os.environ.get("ATTN_TRACE", "0") == "1"
    res = run_bass_kernel_spmd(
        nc, in_maps, core_ids=list(range(NCORES)), trace=trace
    )
    _CACHE["last_results"] = res

    out = np.empty((B, S, D), dtype=np.float32)
    for b in range(B):
        out[b] = res.results[2 * b]["out_part"] + res.results[2 * b + 1]["out_part"]
    return out


# revision 12
# speedup vs baseline: 1.2244x; 1.2216x over previous
"""Multi-head attention (B=4, S=2048, D=1024, H=16) on 8 TRN2 NeuronCores.

Sharding: core c handles batch b = c//2 and head-half hh = c%2 (8 heads).
Each core computes, for its (b, hh):
    QKV^T projection (feature-major layouts, fp32r matmuls on the PE),
    per-head attention  S^T = K^T.T-style layout so softmax-exp feeds the
    PV matmul directly (no on-chip transpose of the attention matrix),
    denominators via a ones-column appended to V (free on the PE),
    partial output projection against its 512 columns of W_out.
Host sums the two head-half partials per batch and stacks batches.
"""

import os

import numpy as np

B, S, D, H = 4, 2048, 1024, 16
HD = D // H  # 64
NCORES = 8
HEADS_PER_CORE = H // 2  # 8
EC = HEADS_PER_CORE * HD  # 512: per-core span of q / k / v features
E_QK = 2 * EC  # 1024 rows of QKV^T kept feature-major (Q then K)
P = 128

# matmul dtype: "f32r" (full-rate fp32 replicated mode), "f32" (4x slower),
# "bf16" (full rate, lower precision)
MM_DTYPE = os.environ.get("ATTN_MM_DTYPE", "f32r")

_CACHE = {}


def _build_nc(s_len=S):
    import concourse.bacc as bacc
    import concourse.bass as bass
    import concourse.mybir as mybir
    import concourse.tile as tile
    from concourse.masks import make_identity

    f32 = mybir.dt.float32
    mm_dt = {
        "f32r": mybir.dt.float32r,
        "f32": mybir.dt.float32,
        "bf16": mybir.dt.bfloat16,
    }[MM_DTYPE]
    # attention (S^T / PV) matmul dtype: partial-array shapes (K=64, M=65)
    # run fp32r at ~2.5 cyc/row on HW, while fp16 streams at 1 cyc/row
    at_dt = {
        "f16": mybir.dt.float16,
        "bf16": mybir.dt.bfloat16,
        "f32r": mybir.dt.float32r,
        "f32": mybir.dt.float32,
    }[os.environ.get("ATTN_ATTN_DT", "f16")]


    S_CHUNKS = s_len // P  # 16
    PANELS = S_CHUNKS // 2  # 8 panels of 256 sequence positions
    QS = s_len // 512  # 4 q-slices of 512
    KC = S_CHUNKS  # 16 k-chunks of 128
    DC = D // P  # 8 contraction chunks for the projections
    ODC = (4 * EC) // P // 4  # 4 contraction chunks (of 128) for out proj

    nc = bacc.Bacc("TRN2", target_bir_lowering=False, debug=False)

    x_ap = nc.dram_tensor("x_b", [s_len, D], f32, kind="ExternalInput").ap()
    wqkv_ap = nc.dram_tensor("w_qkv_s", [3 * EC, D], f32, kind="ExternalInput").ap()
    bqkv_ap = nc.dram_tensor("b_qkv_s", [3 * EC], f32, kind="ExternalInput").ap()
    wout_ap = nc.dram_tensor("w_out_s", [D, EC], f32, kind="ExternalInput").ap()
    bout_ap = nc.dram_tensor("b_out", [D], f32, kind="ExternalInput").ap()
    out_ap = nc.dram_tensor("out_part", [s_len, D], f32, kind="ExternalOutput").ap()

    with tile.TileContext(nc) as tc:
        with tc.tile_pool(name="persist", bufs=1) as persist:
            # persistent SBUF tensors
            # Q^T feature-major; K^T zero-padded to a full 128-row (=d)
            # contraction chunk per head: K=64 matmuls stream at 2 cyc/col
            # on the PE, padded K=128 chunks at 1 cyc/col, and the zero
            # weight rows null out the other head's Q rows in the shared rhs
            qkt = persist.tile([P, DC // 2, s_len], at_dt)
            kpad = persist.tile([P, HEADS_PER_CORE, s_len], at_dt)
            vnat = persist.tile([P, S_CHUNKS, HEADS_PER_CORE, HD + 1], at_dt)
            ident = persist.tile([P, P], f32)
            bqk = persist.tile([P, DC], f32)  # q/k bias, feature on partition
            bv = persist.tile([P, EC], f32)  # v bias broadcast over partitions
            bout = persist.tile([P, D], f32)  # out bias broadcast over partitions
            wout_t = persist.tile([P, ODC, D], mm_dt)

            ones1 = persist.tile([P, 1], f32)
            zeros1 = persist.tile([P, 1], f32)
            make_identity(nc, ident)
            nc.vector.memset(ones1, 1.0)
            nc.vector.memset(zeros1, 0.0)
            for h in range(HEADS_PER_CORE):
                z0 = (1 - h % 2) * HD  # zero the half NOT holding head h
                nc.vector.tensor_copy(
                    kpad[z0 : z0 + HD, h, :], zeros1[0:HD].to_broadcast((HD, s_len))
                )
            for sc in range(S_CHUNKS):
                nc.vector.tensor_copy(
                    vnat[:, sc, :, HD], ones1.to_broadcast((P, HEADS_PER_CORE))
                )
            nc.sync.dma_start(bqk, bqkv_ap[0:E_QK].rearrange("(o p) -> p o", p=P))
            nc.sync.dma_start(bv, bqkv_ap[None, E_QK : 3 * EC].to_broadcast((P, EC)))
            nc.sync.dma_start(bout, bout_ap[None, :].to_broadcast((P, D)))

            # ---- Phase A: transpose weights + x panels, project QKV ----
            with (
                tc.tile_pool(name="wq_t", bufs=1) as wq_pool,
                tc.tile_pool(name="stage", bufs=3) as stage,
                tc.tile_pool(name="xt", bufs=2) as xt_pool,
                tc.tile_pool(name="tp_ps", bufs=2, space="PSUM") as tp_ps,
                tc.tile_pool(name="qk_ps", bufs=3, space="PSUM") as qk_ps,
                tc.tile_pool(name="v_ps", bufs=2, space="PSUM") as v_ps,
            ):
                wqkv_t = wq_pool.tile([P, DC, 3 * EC], mm_dt)
                # W_qkv slice: [1536, 1024] -> W^T [128, 8, 1536]
                for ec in range(3 * EC // P):  # 12
                    wst = stage.tile([P, D], f32, tag="stage")
                    nc.sync.dma_start(wst, wqkv_ap[ec * P : (ec + 1) * P, :])
                    for dc in range(DC):
                        pt = tp_ps.tile([P, P], f32)
                        nc.tensor.transpose(pt, wst[:, dc * P : (dc + 1) * P], ident)
                        nc.vector.tensor_copy(
                            wqkv_t[:, dc, ec * P : (ec + 1) * P], pt
                        )

                for panel in range(PANELS):
                    xt = xt_pool.tile([P, DC, 256], mm_dt, tag="xt")
                    for sc_in in range(2):
                        sc = panel * 2 + sc_in
                        xst = stage.tile([P, D], f32, tag="stage")
                        nc.sync.dma_start(xst, x_ap[sc * P : (sc + 1) * P, :])
                        for dc in range(DC):
                            pt = tp_ps.tile([P, P], f32)
                            nc.tensor.transpose(
                                pt, xst[:, dc * P : (dc + 1) * P], ident
                            )
                            nc.vector.tensor_copy(
                                xt[:, dc, sc_in * P : (sc_in + 1) * P], pt
                            )
                    # Q,K feature-major projection: psum [e-chunk 128, s 256]
                    for ec in range(E_QK // P):  # 8
                        ps = qk_ps.tile([P, 256], f32)
                        for dc in range(DC):
                            nc.tensor.matmul(
                                ps,
                                (wqkv_t[:, dc, ec * P : (ec + 1) * P]),
                                (xt[:, dc, :]),
                                start=(dc == 0),
                                stop=(dc == DC - 1),
                            )
                        sl = slice(panel * 256, (panel + 1) * 256)
                        if ec < E_QK // P // 2:  # Q chunks
                            nc.vector.tensor_scalar_add(
                                qkt[:, ec, sl], ps, bqk[:, ec : ec + 1]
                            )
                        else:  # K chunks: split the two heads into kpad
                            h0 = (ec - E_QK // P // 2) * 2
                            nc.vector.tensor_scalar_add(
                                kpad[0:HD, h0, sl],
                                ps[0:HD, :],
                                bqk[0:HD, ec : ec + 1],
                            )
                            nc.vector.tensor_scalar_add(
                                kpad[HD:P, h0 + 1, sl],
                                ps[HD:P, :],
                                bqk[HD:P, ec : ec + 1],
                            )
                    # V natural: psum [s 128, v 512]
                    for sc_in in range(2):
                        sc = panel * 2 + sc_in
                        ps = v_ps.tile([P, EC], f32)
                        for dc in range(DC):
                            nc.tensor.matmul(
                                ps,
                                (xt[:, dc, sc_in * P : (sc_in + 1) * P]),
                                (wqkv_t[:, dc, E_QK : 3 * EC]),
                                start=(dc == 0),
                                stop=(dc == DC - 1),
                            )
                        for h in range(HEADS_PER_CORE):
                            nc.vector.tensor_add(
                                out=vnat[:, sc, h, 0:HD],
                                in0=ps[:, h * HD : (h + 1) * HD],
                                in1=bv[:, h * HD : (h + 1) * HD],
                            )

            # ---- Phase B: transpose W_out slice -> [128, 4, 1024] ----
            with (
                tc.tile_pool(name="wo_stage", bufs=2) as wo_stage,
                tc.tile_pool(name="tp_ps2", bufs=2, space="PSUM") as tp_ps2,
            ):
                for ec in range(D // P):  # 8
                    wst = wo_stage.tile([P, EC], f32, tag="wo")
                    nc.sync.dma_start(wst, wout_ap[ec * P : (ec + 1) * P, :])
                    for dc in range(ODC):
                        pt = tp_ps2.tile([P, P], f32)
                        nc.tensor.transpose(pt, wst[:, dc * P : (dc + 1) * P], ident)
                        nc.vector.tensor_copy(wout_t[:, dc, ec * P : (ec + 1) * P], pt)

            # ---- Phase C: attention ----
            with tc.tile_pool(name="vt", bufs=1) as vt_pool:
              vt = vt_pool.tile([P, ODC, s_len], mm_dt)  # values^T, d-major
              with (
                tc.tile_pool(name="et", bufs=3) as et_pool,
                tc.tile_pool(name="rb", bufs=2) as rb_pool,
                tc.tile_pool(name="vtt", bufs=2) as vtt_pool,
                tc.tile_pool(name="sg_ps", bufs=2, space="PSUM") as sg_ps,
                tc.tile_pool(name="pv_ps", bufs=2, space="PSUM") as pv_ps,
              ):
                for h in range(HEADS_PER_CORE):
                    qrow = h // 2
                    for qs in range(QS):
                        pv = pv_ps.tile([HD + 1, 512], f32)

                        def pv_mms(kg, ets):
                            # deferred PV for group kg, after S^T of kg+1 has
                            # been emitted — keeps the in-order PE stream from
                            # stalling on the ACT exp and cooling the clock
                            for sub in range(2):
                                kc = kg * 2 + sub
                                nc.tensor.matmul(
                                    pv,
                                    (vnat[:, kc, h, :]),
                                    (ets[kg][:, sub, :]),
                                    start=(kc == 0),
                                    stop=(kc == KC - 1),
                                )

                        ets = {}
                        for kg in range(KC // 2):
                            sg = sg_ps.tile([P, 2, 512], f32)
                            ets[kg] = et_pool.tile(
                                [P, 2, 512], at_dt, tag="et", name="et"
                            )
                            for sub in range(2):
                                kc = kg * 2 + sub
                                nc.tensor.matmul(
                                    sg[:, sub, :],
                                    (kpad[:, h, kc * P : (kc + 1) * P]),
                                    (qkt[:, qrow, qs * 512 : (qs + 1) * 512]),
                                    start=True,
                                    stop=True,
                                )
                            nc.scalar.activation(
                                ets[kg],
                                sg,
                                mybir.ActivationFunctionType.Exp,
                                scale=float(1.0 / np.sqrt(HD)),
                            )
                            if kg >= 1:
                                pv_mms(kg - 1, ets)
                        pv_mms(KC // 2 - 1, ets)
                        # normalize: row HD of pv is the softmax denominator
                        rb = rb_pool.tile([HD + 1, 512], f32, tag="rb")
                        nc.vector.reciprocal(rb[HD : HD + 1, :], pv[HD : HD + 1, :])
                        nc.sync.dma_start(
                            rb[0:HD, :],
                            rb[HD : HD + 1, None, :].to_broadcast((1, HD, 512)),
                        )
                        if h % 2 == 0:
                            nc.vector.tensor_mul(
                                out=vt[0:HD, h // 2, qs * 512 : (qs + 1) * 512],
                                in0=pv[0:HD, :],
                                in1=rb[0:HD, :],
                            )
                        else:
                            vtt = vtt_pool.tile([HD, 512], mm_dt, tag="vtt")
                            nc.vector.tensor_mul(
                                out=vtt, in0=pv[0:HD, :], in1=rb[0:HD, :]
                            )
                            nc.sync.dma_start(
                                vt[HD : 2 * HD, h // 2, qs * 512 : (qs + 1) * 512],
                                vtt,
                            )

              # ---- Phase D: output projection ----
              with (
                    tc.tile_pool(name="osb", bufs=3) as osb_pool,
                    tc.tile_pool(name="op_ps", bufs=3, space="PSUM") as op_ps,
              ):
                    for sc in range(S_CHUNKS):
                        for eh in range(2):
                            ps = op_ps.tile([P, 512], f32)
                            for dc in range(ODC):
                                nc.tensor.matmul(
                                    ps,
                                    (vt[:, dc, sc * P : (sc + 1) * P]),
                                    (wout_t[:, dc, eh * 512 : (eh + 1) * 512]),
                                    start=(dc == 0),
                                    stop=(dc == ODC - 1),
                                )
                            osb = osb_pool.tile([P, 512], f32, tag="osb")
                            nc.vector.tensor_add(
                                out=osb,
                                in0=ps,
                                in1=bout[:, eh * 512 : (eh + 1) * 512],
                            )
                            nc.sync.dma_start(
                                out_ap[
                                    sc * P : (sc + 1) * P, eh * 512 : (eh + 1) * 512
                                ],
                                osb,
                            )

    nc.compile()
    return nc


def _shard(inputs, s_len=S):
    x = np.ascontiguousarray(np.asarray(inputs["x"], dtype=np.float32))
    wqkv = np.asarray(inputs["W_qkv"], dtype=np.float32)
    bqkv = np.asarray(inputs["b_qkv"], dtype=np.float32)
    wout = np.asarray(inputs["W_out"], dtype=np.float32)
    bout = np.asarray(inputs["b_out"], dtype=np.float32)
    zeros_bout = np.zeros_like(bout)

    in_maps = []
    for c in range(NCORES):
        b, hh = c // 2, c % 2
        heads = range(hh * HEADS_PER_CORE, (hh + 1) * HEADS_PER_CORE)
        # reference reshapes qkv to [s, H, 3*HD]: head h owns rows
        # [h*192, h*192+192) of W_qkv as (q 64 | k 64 | v 64)
        rows = np.concatenate(
            [np.arange(h * 3 * HD + t * HD, h * 3 * HD + (t + 1) * HD)
             for t in range(3) for h in heads]
        )
        in_maps.append(
            {
                "x_b": np.ascontiguousarray(x[b, :s_len]),
                "w_qkv_s": np.ascontiguousarray(wqkv[rows]),
                "b_qkv_s": np.ascontiguousarray(bqkv[rows]),
                "w_out_s": np.ascontiguousarray(wout[:, hh * EC : (hh + 1) * EC]),
                "b_out": bout if hh == 0 else zeros_bout,
            }
        )
    return in_maps


def kernel(**inputs) -> np.ndarray:
    from concourse.bass_utils import run_bass_kernel_spmd

    if "nc" not in _CACHE:
        _CACHE["nc"] = _build_nc()
    nc = _CACHE["nc"]

    in_maps = _shard(inputs)
    trace = os.environ.get("ATTN_TRACE", "0") == "1"
    res = run_bass_kernel_spmd(
        nc, in_maps, core_ids=list(range(NCORES)), trace=trace
    )
    _CACHE["last_results"] = res

    out = np.empty((B, S, D), dtype=np.float32)
    for b in range(B):
        out[b] = res.results[2 * b]["out_part"] + res.results[2 * b + 1]["out_part"]
    return out
